# revision 21
# baseline (speedup 1.0000x reference)
"""GCN layer (GCNConv + BatchNorm1d + ReLU + residual) on 8 Trainium2 cores.

Strategy (dst-sharded, batched dma_gather, W applied post-aggregation):
  * Nodes sharded by destination across 8 cores (12500 dst nodes each).
  * Linearity: agg = segsum(norm * x[src]) @ W.T, so the per-edge gather
    fetches RAW x rows (bf16) and W is applied once per 128-dst window
    after aggregation.  No h-table preamble, no AllGather.
  * norm_e = dinv[src]*dinv[dst] is folded into the one-hot selection
    matrix S (S[e, d] = norm_e * [dst_rel_e == d]), so no per-edge scaling
    pass is needed.  norm/drel stream from host (index-derived data only).
  * The per-edge gather uses the batched SWDGE dma_gather (one call moves
    G*128 rows; ~1us fixed + 0.34ns/row) instead of per-128-row
    indirect_dma_start calls (~1.1us EACH, the old bottleneck).
  * int16 gather indices only reach 32767, so the x table is split into 4
    chunks of 25088 rows; each core's edge list is bucketed by
    (dst_window, src_chunk) and padded to 128-edge blocks.
  * Per window: psum[i,d] += gathered_block.T @ S_block; then
    psum2[o,d] = W.T-matmul; evict with fused BN-stat accumulation.
  * BN stats via tiny [128,2] AllReduce; affine+ReLU+residual epilogue.
"""

import math
from contextlib import ExitStack

import numpy as np

P = 128
BN_EPS = 1e-5

N_FULL = 100000
N_CORES = 8
N_LOC = N_FULL // N_CORES  # 12500
WW = 64  # dst-window width (S matrix columns; narrower = cheaper DVE S build)
N_WIN = math.ceil(N_LOC / WW)  # 196
N_PAD = N_WIN * WW  # 12544
N_CHUNK = 4
CHUNK = 25088  # x-table rows per chunk (fits int16 indices)
NX = N_CHUNK * CHUNK  # 100352 padded x rows
G = 8  # blocks (of 128 edges) per dma_gather call (ucode cap: 1024 idxs)
KB = 16  # blocks per batched S build


# ---------------------------------------------------------------------------
# Host-side index preprocessing (index-derived data only; all tensor math
# happens on device).
# ---------------------------------------------------------------------------
def make_plan(edge_index: np.ndarray):
    import ml_dtypes

    src = np.asarray(edge_index[0], dtype=np.int64)
    dst = np.asarray(edge_index[1], dtype=np.int64)

    # self-loops count toward deg but are NOT gathered: their contribution
    # dinv[n]^2 * x[n] is added from SBUF at eviction time
    deg = (np.bincount(dst, minlength=N_FULL) + 1).astype(np.float64)
    dinv = 1.0 / np.sqrt(deg)
    norm_all = (dinv[src] * dinv[dst]).astype(np.float32)
    dinv2 = (dinv * dinv).astype(np.float32)

    core = dst // N_LOC
    dloc = dst - core * N_LOC
    win = dloc // WW
    drel_all = (dloc % WW).astype(np.float32)
    ch = src // CHUNK
    sidx_all = (src - ch * CHUNK).astype(np.int16)

    order = np.lexsort((src, ch, win, core))
    key = (core * N_WIN + win) * N_CHUNK + ch
    cnt = np.bincount(key, minlength=N_CORES * N_WIN * N_CHUNK).reshape(
        N_CORES, N_WIN * N_CHUNK
    )
    # shared SPMD layout: run padded only to the max count over cores (no
    # 128-alignment).  Blocks straddle window boundaries; each (window,
    # chunk, block) segment gets its own masked S column (foreign lanes
    # have norm=0 and contribute nothing to the full-128-lane matmul).
    run_m = cnt.max(axis=0).astype(np.int64)  # [N_WIN*N_CHUNK]
    run_off = np.zeros(N_WIN * N_CHUNK, dtype=np.int64)
    slen = np.zeros(N_CHUNK, dtype=np.int64)
    for c in range(N_CHUNK):
        ids = np.arange(N_WIN) * N_CHUNK + c
        m = run_m[ids]
        run_off[ids] = np.concatenate([[0], np.cumsum(m[:-1])])
        slen[c] = m.sum()
    bc = [-(-int(slen[c]) // P) for c in range(N_CHUNK)]

    # segment enumeration in window-major (= consumption) order
    seg_base = np.zeros(N_WIN * N_CHUNK + 1, dtype=np.int64)
    schedule = []
    for w in range(N_WIN):
        segs = []
        for c in range(N_CHUNK):
            r = w * N_CHUNK + c
            m = int(run_m[r])
            if m == 0:
                seg_base[r + 1] = seg_base[r]
                continue
            o = int(run_off[r])
            fb, lb = o >> 7, (o + m - 1) >> 7
            seg_base[r + 1] = seg_base[r] + (lb - fb + 1)
            for b in range(fb, lb + 1):
                segs.append((c, b, int(seg_base[r]) + b - fb))
        schedule.append(segs)
    n_seg = int(seg_base[-1])

    # per-core slot arrays (indexed by segment, not block)
    drel_arr = np.zeros((N_CORES, P, n_seg), dtype=np.float32)
    norm_arr = np.zeros((N_CORES, P, n_seg), dtype=np.float32)
    idx_arr = [np.zeros((N_CORES, bc[c] * P), dtype=np.int16) for c in range(N_CHUNK)]

    core_s = core[order]
    seg = np.searchsorted(core_s, np.arange(N_CORES + 1))
    for k in range(N_CORES):
        e = order[seg[k] : seg[k + 1]]
        run_id = win[e] * N_CHUNK + ch[e]
        run_lo = np.concatenate([[0], np.cumsum(cnt[k])]).astype(np.int64)
        j = np.arange(len(e)) - run_lo[run_id]
        slot = run_off[run_id] + j
        blk = slot >> 7
        lane = slot & 127
        sid = seg_base[run_id] + blk - (run_off[run_id] >> 7)
        drel_arr[k, lane, sid] = drel_all[e]
        norm_arr[k, lane, sid] = norm_all[e]
        for c in range(N_CHUNK):
            m = ch[e] == c
            idx_arr[c][k, slot[m]] = sidx_all[e[m]]

    # wrap indices: element i -> [i%16, i//16], replicated to 128 partitions
    idx_wrapped = []
    for c in range(N_CHUNK):
        a = idx_arr[c].reshape(N_CORES, bc[c] * P // 16, 16)
        a = np.ascontiguousarray(np.transpose(a, (0, 2, 1)))  # [cores, 16, L]
        idx_wrapped.append(np.tile(a, (1, 8, 1)))  # [cores, 128, L]

    d2 = np.zeros((N_CORES, N_PAD), dtype=np.float32)
    for k in range(N_CORES):
        d2[k, :N_LOC] = dinv2[k * N_LOC : (k + 1) * N_LOC]

    return dict(
        n_seg=n_seg,
        bc=bc,
        schedule=schedule,
        drel=drel_arr.astype(ml_dtypes.bfloat16),
        norm=norm_arr.astype(ml_dtypes.bfloat16),
        idx=idx_wrapped,
        dinv2=d2.astype(ml_dtypes.bfloat16),
    )


# ---------------------------------------------------------------------------
# Device program
# ---------------------------------------------------------------------------
def build_nc(plan):
    import concourse.bacc as bacc
    import concourse.mybir as mybir
    import concourse.tile as tile
    from concourse.ap import AP

    f32 = mybir.dt.float32
    bf16 = mybir.dt.bfloat16
    i16 = mybir.dt.int16
    AF = mybir.ActivationFunctionType
    OP = mybir.AluOpType

    n_seg = plan["n_seg"]
    bc = plan["bc"]
    schedule = plan["schedule"]

    nc = bacc.Bacc(
        "TRN2",
        target_bir_lowering=False,
        debug=False,
        num_devices=N_CORES,
        num_swdge_queues=4,
    )

    xq = nc.dram_tensor("xq", [NX, P], bf16, kind="ExternalInput")
    xres = nc.dram_tensor("xres", [P, N_PAD], bf16, kind="ExternalInput")
    wt = nc.dram_tensor("wt", [P, P], f32, kind="ExternalInput")
    iota_in = nc.dram_tensor("iota", [P, P], bf16, kind="ExternalInput")
    gam = nc.dram_tensor("gam", [P, 1], f32, kind="ExternalInput")
    bet = nc.dram_tensor("bet", [P, 1], f32, kind="ExternalInput")
    idx_d = [
        nc.dram_tensor(f"idx{c}", [P, bc[c] * 8], i16, kind="ExternalInput")
        for c in range(N_CHUNK)
    ]
    dinv2_d = nc.dram_tensor("dinv2", [P, N_PAD], bf16, kind="ExternalInput")
    drel_d = nc.dram_tensor("drel", [P, n_seg], bf16, kind="ExternalInput")
    norm_d = nc.dram_tensor("norm", [P, n_seg], bf16, kind="ExternalInput")
    out_d = nc.dram_tensor("out", [P, N_PAD], bf16, kind="ExternalOutput")

    rg = [list(range(N_CORES))]

    with tile.TileContext(nc) as tc, ExitStack() as ctx:
        const = ctx.enter_context(tc.tile_pool(name="const", bufs=1))
        gat = ctx.enter_context(tc.tile_pool(name="gat", bufs=3))
        sbld = ctx.enter_context(tc.tile_pool(name="sbld", bufs=3))
        work = ctx.enter_context(tc.tile_pool(name="work", bufs=4))
        win_ps = ctx.enter_context(tc.tile_pool(name="win_ps", bufs=2, space="PSUM"))
        out_ps = ctx.enter_context(tc.tile_pool(name="out_ps", bufs=2, space="PSUM"))
        dram = ctx.enter_context(tc.tile_pool(name="dram", bufs=1, space="DRAM"))

        # ---- constants / streams resident in SBUF
        iota_sb = const.tile([P, P], bf16)
        nc.sync.dma_start(out=iota_sb[:], in_=iota_in[:, :])
        gam_sb = const.tile([P, 1], f32)
        nc.sync.dma_start(out=gam_sb[:], in_=gam[:, :])
        bet_sb = const.tile([P, 1], f32)
        nc.sync.dma_start(out=bet_sb[:], in_=bet[:, :])
        wt_sb = const.tile([P, P], f32)
        nc.sync.dma_start(out=wt_sb[:], in_=wt[:, :])
        wt_bf = const.tile([P, P], bf16)
        nc.vector.tensor_copy(wt_bf[:], wt_sb[:])
        xres_sb = const.tile([P, N_PAD], bf16)
        nc.sync.dma_start(out=xres_sb[:], in_=xres[:, :])
        # xd = dinv^2 * x (bf16, in place): the self-loop contribution
        xd_sb = const.tile([P, N_PAD], bf16)
        nc.sync.dma_start(out=xd_sb[:], in_=dinv2_d[:, :])
        nc.vector.tensor_tensor(out=xd_sb[:], in0=xd_sb[:], in1=xres_sb[:], op=OP.mult)
        drel_sb = const.tile([P, n_seg], bf16)
        nc.sync.dma_start(out=drel_sb[:], in_=drel_d[:, :])
        norm_sb = const.tile([P, n_seg], bf16)
        nc.sync.dma_start(out=norm_sb[:], in_=norm_d[:, :])
        idx_sb = []
        for c in range(N_CHUNK):
            t = const.tile([P, bc[c] * 8], i16, name=f"idxsb{c}", tag=f"idxsb{c}")
            nc.sync.dma_start(out=t[:], in_=idx_d[c][:, :])
            idx_sb.append(t)

        agg_out = const.tile([P, N_PAD], bf16)
        sum_c = const.tile([P, N_WIN], f32)
        sq_c = const.tile([P, N_WIN], f32)

        # ---- gather call / S-batch emission helpers
        ncalls = [-(-bc[c] // G) for c in range(N_CHUNK)]
        gt_tiles = [dict() for _ in range(N_CHUNK)]
        issued = [0] * N_CHUNK
        s_tiles = {}
        n_sbatch = -(-n_seg // KB)
        built = [0]

        def issue_call(c):
            q = issued[c]
            nb = min(G, bc[c] - q * G)
            t = gat.tile([P, nb * P], bf16, tag=f"gt{c}", bufs=5)
            nc.gpsimd.dma_gather(
                t[:].rearrange("p (b e) -> p b e", e=P),
                xq[c * CHUNK : (c + 1) * CHUNK, :],
                idx_sb[c][:, q * G * 8 : q * G * 8 + nb * 8],
                nb * P,
                nb * P,
                P,
                single_packet=False,
                queue_num=c,
            )
            gt_tiles[c][q] = t
            issued[c] = q + 1

        def build_sbatch():
            sb = built[0]
            kb = min(KB, n_seg - sb * KB)
            t0 = sbld.tile([P, kb * WW], bf16, tag="t0")
            iota_ap = iota_sb[:, 0:WW]
            iota3 = AP(
                iota_ap.tensor,
                iota_ap.offset,
                [list(iota_ap.ap[0]), [0, kb], list(iota_ap.ap[1])],
            )
            nc.vector.tensor_tensor(
                out=t0[:].rearrange("p (b d) -> p b d", d=WW),
                in0=drel_sb[:, sb * KB : sb * KB + kb].to_broadcast([P, kb, WW]),
                in1=iota3,
                op=OP.is_equal,
            )
            st = sbld.tile([P, kb * WW], bf16, tag="st")
            nc.vector.tensor_tensor(
                out=st[:].rearrange("p (b d) -> p b d", d=WW),
                in0=t0[:].rearrange("p (b d) -> p b d", d=WW),
                in1=norm_sb[:, sb * KB : sb * KB + kb].to_broadcast([P, kb, WW]),
                op=OP.mult,
            )
            s_tiles[sb] = st
            built[0] = sb + 1

        for c in range(N_CHUNK):
            if ncalls[c] > 0:
                issue_call(c)
        build_sbatch()

        # ---- main loop: aggregation matmuls + per-window eviction
        for w in range(N_WIN):
            segs = schedule[w]
            wp = win_ps.tile([P, WW], f32, tag="win")
            for si, (c, b, sid) in enumerate(segs):
                q = b // G
                while issued[c] <= min(q + 2, ncalls[c] - 1):
                    issue_call(c)
                sb = sid // KB
                while built[0] <= min(sb + 2, n_sbatch - 1):
                    build_sbatch()
                gt = gt_tiles[c][q]
                st = s_tiles[sb]
                nc.tensor.matmul(
                    out=wp[:],
                    lhsT=gt[:, (b - q * G) * P : (b - q * G + 1) * P],
                    rhs=st[:, (sid - sb * KB) * WW : (sid - sb * KB + 1) * WW],
                    start=(si == 0),
                    stop=(si == len(segs) - 1),
                )

            # evict: add the self-loop term, apply W, accumulate BN stats
            agg_i = work.tile([P, WW], bf16, tag="agg_i")
            nc.vector.tensor_tensor(
                out=agg_i[:],
                in0=xd_sb[:, w * WW : (w + 1) * WW],
                in1=wp[:],
                op=OP.add,
            )
            ps2 = out_ps.tile([P, WW], f32, tag="ps2")
            nc.tensor.matmul(
                out=ps2[:], lhsT=wt_bf[:], rhs=agg_i[:], start=True, stop=True
            )
            nc.scalar.activation(
                out=agg_out[:, w * WW : (w + 1) * WW], in_=ps2[:], func=AF.Copy
            )
            nc.vector.tensor_reduce(
                out=sum_c[:, w : w + 1],
                in_=ps2[:],
                axis=mybir.AxisListType.X,
                op=OP.add,
            )
            sqt = work.tile([P, WW], f32, tag="sqt")
            nc.scalar.activation(
                out=sqt[:],
                in_=ps2[:],
                func=AF.Square,
                accum_out=sq_c[:, w : w + 1],
            )

        # ---- BN statistics all-reduce
        stot = const.tile([P, 2], f32)
        nc.vector.tensor_reduce(
            out=stot[:, 0:1], in_=sum_c[:], axis=mybir.AxisListType.X, op=OP.add
        )
        nc.vector.tensor_reduce(
            out=stot[:, 1:2], in_=sq_c[:], axis=mybir.AxisListType.X, op=OP.add
        )
        stats_l = dram.tile([P, 2], f32)
        stats_g = dram.tile([P, 2], f32)
        nc.sync.dma_start(out=stats_l[:, :], in_=stot[:])
        nc.gpsimd.collective_compute(
            "AllReduce",
            mybir.AluOpType.add,
            replica_groups=rg,
            ins=[stats_l[:].opt()],
            outs=[stats_g[:].opt()],
        )
        sg = const.tile([P, 2], f32)
        nc.sync.dma_start(out=sg[:], in_=stats_g[:, :])

        # ---- BN affine params: s = gamma/std, t = beta - mean*s
        mean = const.tile([P, 1], f32)
        nc.vector.tensor_scalar_mul(mean[:], sg[:, 0:1], 1.0 / N_FULL)
        var = const.tile([P, 1], f32)
        nc.vector.tensor_scalar_mul(var[:], sg[:, 1:2], 1.0 / N_FULL)
        msq = const.tile([P, 1], f32)
        nc.vector.tensor_mul(msq[:], mean[:], mean[:])
        nc.vector.tensor_sub(var[:], var[:], msq[:])
        nc.vector.tensor_scalar_add(var[:], var[:], BN_EPS)
        nc.scalar.sqrt(var[:], var[:])
        s_t = const.tile([P, 1], f32)
        nc.vector.reciprocal(s_t[:], var[:])
        nc.vector.tensor_mul(s_t[:], gam_sb[:], s_t[:])
        t_t = const.tile([P, 1], f32)
        nc.vector.tensor_mul(t_t[:], mean[:], s_t[:])
        nc.vector.tensor_sub(t_t[:], bet_sb[:], t_t[:])

        # ---- epilogue: out = relu(agg*s + t) + x.  The BN affine is
        # per-feature (per-partition), so the whole node range is processed
        # in a few wide strips, written back into agg_out (bf16), then
        # stored with one large DMA.
        NSTRIP = 8
        SW = N_PAD // NSTRIP
        for s in range(NSTRIP):
            y = work.tile([P, SW], f32, tag="y")
            nc.scalar.activation(
                out=y[:],
                in_=agg_out[:, s * SW : (s + 1) * SW],
                func=AF.Relu,
                scale=s_t[:],
                bias=t_t[:],
            )
            nc.vector.tensor_tensor(
                out=agg_out[:, s * SW : (s + 1) * SW],
                in0=y[:],
                in1=xres_sb[:, s * SW : (s + 1) * SW],
                op=OP.add,
            )
        nc.sync.dma_start(out=out_d[:, :], in_=agg_out[:])

    nc.compile()
    return nc


# ---------------------------------------------------------------------------
# Host wrapper
# ---------------------------------------------------------------------------
def _in_maps(plan, x, W, gamma, beta):
    import ml_dtypes

    x = np.asarray(x, dtype=np.float32)
    xq = np.zeros((NX, P), dtype=ml_dtypes.bfloat16)
    xq[:N_FULL] = x.astype(ml_dtypes.bfloat16)
    wt = np.ascontiguousarray(np.asarray(W, dtype=np.float32).T)
    iota = np.tile(np.arange(P, dtype=ml_dtypes.bfloat16), (P, 1))
    gam = np.asarray(gamma, dtype=np.float32).reshape(P, 1)
    bet = np.asarray(beta, dtype=np.float32).reshape(P, 1)

    maps = []
    for k in range(N_CORES):
        xres = np.zeros((P, N_PAD), dtype=ml_dtypes.bfloat16)
        xres[:, :N_LOC] = x[k * N_LOC : (k + 1) * N_LOC].T.astype(ml_dtypes.bfloat16)
        m = dict(
            xq=xq,
            xres=xres,
            wt=wt,
            iota=iota,
            gam=gam,
            bet=bet,
            drel=np.ascontiguousarray(plan["drel"][k]),
            norm=np.ascontiguousarray(plan["norm"][k]),
            dinv2=np.ascontiguousarray(np.tile(plan["dinv2"][k], (P, 1))),
        )
        for c in range(N_CHUNK):
            m[f"idx{c}"] = np.ascontiguousarray(plan["idx"][c][k])
        maps.append(m)
    return maps


def run(x, edge_index, W, b, gamma, beta, trace=False):
    from concourse.bass_utils import run_bass_kernel_spmd

    plan = make_plan(np.asarray(edge_index))
    nc = build_nc(plan)
    maps = _in_maps(plan, x, W, gamma, beta)
    res = run_bass_kernel_spmd(nc, maps, core_ids=list(range(N_CORES)), trace=trace)
    out = np.concatenate(
        [res.results[k]["out"].astype(np.float32).T[:N_LOC] for k in range(N_CORES)],
        axis=0,
    )
    return out, res


def kernel(x, edge_index, W, b, gamma, beta):
    out, _ = run(x, edge_index, W, b, gamma, beta)
    return out


# revision 22
# speedup vs baseline: 1.0537x; 1.0537x over previous
"""GCN layer (GCNConv + BatchNorm1d + ReLU + residual) on 8 Trainium2 cores.

Strategy (dst-sharded, batched dma_gather, W applied post-aggregation):
  * Nodes sharded by destination across 8 cores (12500 dst nodes each).
  * Linearity: agg = segsum(norm * x[src]) @ W.T, so the per-edge gather
    fetches RAW x rows (bf16) and W is applied once per 128-dst window
    after aggregation.  No h-table preamble, no AllGather.
  * norm_e = dinv[src]*dinv[dst] is folded into the one-hot selection
    matrix S (S[e, d] = norm_e * [dst_rel_e == d]), so no per-edge scaling
    pass is needed.  norm/drel stream from host (index-derived data only).
  * The per-edge gather uses the batched SWDGE dma_gather (one call moves
    G*128 rows; ~1us fixed + 0.34ns/row) instead of per-128-row
    indirect_dma_start calls (~1.1us EACH, the old bottleneck).
  * int16 gather indices only reach 32767, so the x table is split into 4
    chunks of 25088 rows; each core's edge list is bucketed by
    (dst_window, src_chunk) and padded to 128-edge blocks.
  * Per window: psum[i,d] += gathered_block.T @ S_block; then
    psum2[o,d] = W.T-matmul; evict with fused BN-stat accumulation.
  * BN stats via tiny [128,2] AllReduce; affine+ReLU+residual epilogue.
"""

import math
from contextlib import ExitStack

import numpy as np

P = 128
BN_EPS = 1e-5

N_FULL = 100000
N_CORES = 8
N_LOC = N_FULL // N_CORES  # 12500
WW = 64  # dst-window width (S matrix columns; narrower = cheaper DVE S build)
N_WIN = math.ceil(N_LOC / WW)  # 196
N_PAD = N_WIN * WW  # 12544
N_CHUNK = 4
CHUNK = 25088  # x-table rows per chunk (fits int16 indices)
NX = N_CHUNK * CHUNK  # 100352 padded x rows
G = 8  # blocks (of 128 edges) per dma_gather call (ucode cap: 1024 idxs)
KB = 16  # blocks per batched S build


# ---------------------------------------------------------------------------
# Host-side index preprocessing (index-derived data only; all tensor math
# happens on device).
# ---------------------------------------------------------------------------
def make_plan(edge_index: np.ndarray):
    import ml_dtypes

    src = np.asarray(edge_index[0], dtype=np.int64)
    dst = np.asarray(edge_index[1], dtype=np.int64)

    # self-loops count toward deg but are NOT gathered: their contribution
    # dinv[n]^2 * x[n] is added from SBUF at eviction time
    deg = (np.bincount(dst, minlength=N_FULL) + 1).astype(np.float64)
    dinv = 1.0 / np.sqrt(deg)
    norm_all = (dinv[src] * dinv[dst]).astype(np.float32)
    dinv2 = (dinv * dinv).astype(np.float32)

    core = dst // N_LOC
    dloc = dst - core * N_LOC
    win = dloc // WW
    drel_all = (dloc % WW).astype(np.float32)
    ch = src // CHUNK
    sidx_all = (src - ch * CHUNK).astype(np.int16)

    order = np.lexsort((src, ch, win, core))
    key = (core * N_WIN + win) * N_CHUNK + ch
    cnt = np.bincount(key, minlength=N_CORES * N_WIN * N_CHUNK).reshape(
        N_CORES, N_WIN * N_CHUNK
    )
    # shared SPMD layout: run padded only to the max count over cores (no
    # 128-alignment).  Blocks straddle window boundaries; each (window,
    # chunk, block) segment gets its own masked S column (foreign lanes
    # have norm=0 and contribute nothing to the full-128-lane matmul).
    run_m = cnt.max(axis=0).astype(np.int64)  # [N_WIN*N_CHUNK]
    run_off = np.zeros(N_WIN * N_CHUNK, dtype=np.int64)
    slen = np.zeros(N_CHUNK, dtype=np.int64)
    for c in range(N_CHUNK):
        ids = np.arange(N_WIN) * N_CHUNK + c
        m = run_m[ids]
        run_off[ids] = np.concatenate([[0], np.cumsum(m[:-1])])
        slen[c] = m.sum()
    bc = [-(-int(slen[c]) // P) for c in range(N_CHUNK)]

    # segment enumeration in window-major (= consumption) order
    seg_base = np.zeros(N_WIN * N_CHUNK + 1, dtype=np.int64)
    schedule = []
    for w in range(N_WIN):
        segs = []
        for c in range(N_CHUNK):
            r = w * N_CHUNK + c
            m = int(run_m[r])
            if m == 0:
                seg_base[r + 1] = seg_base[r]
                continue
            o = int(run_off[r])
            fb, lb = o >> 7, (o + m - 1) >> 7
            seg_base[r + 1] = seg_base[r] + (lb - fb + 1)
            for b in range(fb, lb + 1):
                segs.append((c, b, int(seg_base[r]) + b - fb))
        schedule.append(segs)
    n_seg = int(seg_base[-1])

    # per-core slot arrays (indexed by segment, not block)
    drel_arr = np.zeros((N_CORES, P, n_seg), dtype=np.float32)
    norm_arr = np.zeros((N_CORES, P, n_seg), dtype=np.float32)
    idx_arr = [np.zeros((N_CORES, bc[c] * P), dtype=np.int16) for c in range(N_CHUNK)]

    core_s = core[order]
    seg = np.searchsorted(core_s, np.arange(N_CORES + 1))
    for k in range(N_CORES):
        e = order[seg[k] : seg[k + 1]]
        run_id = win[e] * N_CHUNK + ch[e]
        run_lo = np.concatenate([[0], np.cumsum(cnt[k])]).astype(np.int64)
        j = np.arange(len(e)) - run_lo[run_id]
        slot = run_off[run_id] + j
        blk = slot >> 7
        lane = slot & 127
        sid = seg_base[run_id] + blk - (run_off[run_id] >> 7)
        drel_arr[k, lane, sid] = drel_all[e]
        norm_arr[k, lane, sid] = norm_all[e]
        for c in range(N_CHUNK):
            m = ch[e] == c
            idx_arr[c][k, slot[m]] = sidx_all[e[m]]

    # wrap indices: element i -> [i%16, i//16], replicated to 128 partitions
    idx_wrapped = []
    for c in range(N_CHUNK):
        a = idx_arr[c].reshape(N_CORES, bc[c] * P // 16, 16)
        a = np.ascontiguousarray(np.transpose(a, (0, 2, 1)))  # [cores, 16, L]
        idx_wrapped.append(np.tile(a, (1, 8, 1)))  # [cores, 128, L]

    d2 = np.zeros((N_CORES, N_PAD), dtype=np.float32)
    for k in range(N_CORES):
        d2[k, :N_LOC] = dinv2[k * N_LOC : (k + 1) * N_LOC]

    return dict(
        n_seg=n_seg,
        bc=bc,
        schedule=schedule,
        drel=drel_arr.astype(ml_dtypes.bfloat16),
        norm=norm_arr.astype(ml_dtypes.bfloat16),
        idx=idx_wrapped,
        dinv2=d2.astype(ml_dtypes.bfloat16),
    )


# ---------------------------------------------------------------------------
# Device program
# ---------------------------------------------------------------------------
def build_nc(plan):
    import concourse.bacc as bacc
    import concourse.mybir as mybir
    import concourse.tile as tile
    from concourse.ap import AP

    f32 = mybir.dt.float32
    bf16 = mybir.dt.bfloat16
    i16 = mybir.dt.int16
    AF = mybir.ActivationFunctionType
    OP = mybir.AluOpType

    n_seg = plan["n_seg"]
    bc = plan["bc"]
    schedule = plan["schedule"]

    nc = bacc.Bacc(
        "TRN2",
        target_bir_lowering=False,
        debug=False,
        num_devices=N_CORES,
        num_swdge_queues=4,
    )

    xq = nc.dram_tensor("xq", [NX, P], bf16, kind="ExternalInput")
    xres = nc.dram_tensor("xres", [P, N_PAD], bf16, kind="ExternalInput")
    wt = nc.dram_tensor("wt", [P, P], f32, kind="ExternalInput")
    iota_in = nc.dram_tensor("iota", [P, P], bf16, kind="ExternalInput")
    gam = nc.dram_tensor("gam", [P, 1], f32, kind="ExternalInput")
    bet = nc.dram_tensor("bet", [P, 1], f32, kind="ExternalInput")
    idx_d = [
        nc.dram_tensor(f"idx{c}", [P, bc[c] * 8], i16, kind="ExternalInput")
        for c in range(N_CHUNK)
    ]
    dinv2_d = nc.dram_tensor("dinv2", [P, N_PAD], bf16, kind="ExternalInput")
    drel_d = nc.dram_tensor("drel", [P, n_seg], bf16, kind="ExternalInput")
    norm_d = nc.dram_tensor("norm", [P, n_seg], bf16, kind="ExternalInput")
    out_d = nc.dram_tensor("out", [P, N_PAD], bf16, kind="ExternalOutput")

    rg = [list(range(N_CORES))]

    with tile.TileContext(nc) as tc, ExitStack() as ctx:
        const = ctx.enter_context(tc.tile_pool(name="const", bufs=1))
        gat = ctx.enter_context(tc.tile_pool(name="gat", bufs=3))
        sbld = ctx.enter_context(tc.tile_pool(name="sbld", bufs=3))
        work = ctx.enter_context(tc.tile_pool(name="work", bufs=4))
        win_ps = ctx.enter_context(tc.tile_pool(name="win_ps", bufs=2, space="PSUM"))
        out_ps = ctx.enter_context(tc.tile_pool(name="out_ps", bufs=2, space="PSUM"))
        dram = ctx.enter_context(tc.tile_pool(name="dram", bufs=1, space="DRAM"))

        # ---- constants / streams resident in SBUF
        iota_sb = const.tile([P, P], bf16)
        nc.sync.dma_start(out=iota_sb[:], in_=iota_in[:, :])
        gam_sb = const.tile([P, 1], f32)
        nc.sync.dma_start(out=gam_sb[:], in_=gam[:, :])
        bet_sb = const.tile([P, 1], f32)
        nc.sync.dma_start(out=bet_sb[:], in_=bet[:, :])
        wt_sb = const.tile([P, P], f32)
        nc.sync.dma_start(out=wt_sb[:], in_=wt[:, :])
        wt_bf = const.tile([P, P], bf16)
        nc.vector.tensor_copy(wt_bf[:], wt_sb[:])
        xres_sb = const.tile([P, N_PAD], bf16)
        nc.sync.dma_start(out=xres_sb[:], in_=xres[:, :])
        # xd = dinv^2 * x (bf16, in place): the self-loop contribution
        xd_sb = const.tile([P, N_PAD], bf16)
        nc.sync.dma_start(out=xd_sb[:], in_=dinv2_d[:, :])
        nc.vector.tensor_tensor(out=xd_sb[:], in0=xd_sb[:], in1=xres_sb[:], op=OP.mult)
        drel_sb = const.tile([P, n_seg], bf16)
        nc.sync.dma_start(out=drel_sb[:], in_=drel_d[:, :])
        norm_sb = const.tile([P, n_seg], bf16)
        nc.sync.dma_start(out=norm_sb[:], in_=norm_d[:, :])
        idx_sb = []
        for c in range(N_CHUNK):
            t = const.tile([P, bc[c] * 8], i16, name=f"idxsb{c}", tag=f"idxsb{c}")
            nc.sync.dma_start(out=t[:], in_=idx_d[c][:, :])
            idx_sb.append(t)

        agg_out = const.tile([P, N_PAD], bf16)
        sum_c = const.tile([P, N_WIN], f32)
        sq_c = const.tile([P, N_WIN], f32)

        # ---- gather call / S-batch emission helpers
        ncalls = [-(-bc[c] // G) for c in range(N_CHUNK)]
        gt_tiles = [dict() for _ in range(N_CHUNK)]
        issued = [0] * N_CHUNK
        s_tiles = {}
        n_sbatch = -(-n_seg // KB)
        built = [0]

        def issue_call(c):
            q = issued[c]
            nb = min(G, bc[c] - q * G)
            t = gat.tile([P, nb * P], bf16, tag=f"gt{c}", bufs=5)
            nc.gpsimd.dma_gather(
                t[:].rearrange("p (b e) -> p b e", e=P),
                xq[c * CHUNK : (c + 1) * CHUNK, :],
                idx_sb[c][:, q * G * 8 : q * G * 8 + nb * 8],
                nb * P,
                nb * P,
                P,
                queue_num=c,
            )
            gt_tiles[c][q] = t
            issued[c] = q + 1

        def build_sbatch():
            sb = built[0]
            kb = min(KB, n_seg - sb * KB)
            t0 = sbld.tile([P, kb * WW], bf16, tag="t0")
            iota_ap = iota_sb[:, 0:WW]
            iota3 = AP(
                iota_ap.tensor,
                iota_ap.offset,
                [list(iota_ap.ap[0]), [0, kb], list(iota_ap.ap[1])],
            )
            nc.vector.tensor_tensor(
                out=t0[:].rearrange("p (b d) -> p b d", d=WW),
                in0=drel_sb[:, sb * KB : sb * KB + kb].to_broadcast([P, kb, WW]),
                in1=iota3,
                op=OP.is_equal,
            )
            st = sbld.tile([P, kb * WW], bf16, tag="st")
            nc.vector.tensor_tensor(
                out=st[:].rearrange("p (b d) -> p b d", d=WW),
                in0=t0[:].rearrange("p (b d) -> p b d", d=WW),
                in1=norm_sb[:, sb * KB : sb * KB + kb].to_broadcast([P, kb, WW]),
                op=OP.mult,
            )
            s_tiles[sb] = st
            built[0] = sb + 1

        for c in range(N_CHUNK):
            if ncalls[c] > 0:
                issue_call(c)
        build_sbatch()

        # ---- main loop: aggregation matmuls + per-window eviction
        for w in range(N_WIN):
            segs = schedule[w]
            wp = win_ps.tile([P, WW], f32, tag="win")
            for si, (c, b, sid) in enumerate(segs):
                q = b // G
                while issued[c] <= min(q + 2, ncalls[c] - 1):
                    issue_call(c)
                sb = sid // KB
                while built[0] <= min(sb + 2, n_sbatch - 1):
                    build_sbatch()
                gt = gt_tiles[c][q]
                st = s_tiles[sb]
                nc.tensor.matmul(
                    out=wp[:],
                    lhsT=gt[:, (b - q * G) * P : (b - q * G + 1) * P],
                    rhs=st[:, (sid - sb * KB) * WW : (sid - sb * KB + 1) * WW],
                    start=(si == 0),
                    stop=(si == len(segs) - 1),
                )

            # evict: add the self-loop term, apply W, accumulate BN stats
            agg_i = work.tile([P, WW], bf16, tag="agg_i")
            nc.vector.tensor_tensor(
                out=agg_i[:],
                in0=xd_sb[:, w * WW : (w + 1) * WW],
                in1=wp[:],
                op=OP.add,
            )
            ps2 = out_ps.tile([P, WW], f32, tag="ps2")
            nc.tensor.matmul(
                out=ps2[:], lhsT=wt_bf[:], rhs=agg_i[:], start=True, stop=True
            )
            nc.scalar.activation(
                out=agg_out[:, w * WW : (w + 1) * WW], in_=ps2[:], func=AF.Copy
            )
            nc.vector.tensor_reduce(
                out=sum_c[:, w : w + 1],
                in_=ps2[:],
                axis=mybir.AxisListType.X,
                op=OP.add,
            )
            sqt = work.tile([P, WW], f32, tag="sqt")
            nc.scalar.activation(
                out=sqt[:],
                in_=ps2[:],
                func=AF.Square,
                accum_out=sq_c[:, w : w + 1],
            )

        # ---- BN statistics all-reduce
        stot = const.tile([P, 2], f32)
        nc.vector.tensor_reduce(
            out=stot[:, 0:1], in_=sum_c[:], axis=mybir.AxisListType.X, op=OP.add
        )
        nc.vector.tensor_reduce(
            out=stot[:, 1:2], in_=sq_c[:], axis=mybir.AxisListType.X, op=OP.add
        )
        stats_l = dram.tile([P, 2], f32)
        stats_g = dram.tile([P, 2], f32)
        nc.sync.dma_start(out=stats_l[:, :], in_=stot[:])
        nc.gpsimd.collective_compute(
            "AllReduce",
            mybir.AluOpType.add,
            replica_groups=rg,
            ins=[stats_l[:].opt()],
            outs=[stats_g[:].opt()],
        )
        sg = const.tile([P, 2], f32)
        nc.sync.dma_start(out=sg[:], in_=stats_g[:, :])

        # ---- BN affine params: s = gamma/std, t = beta - mean*s
        mean = const.tile([P, 1], f32)
        nc.vector.tensor_scalar_mul(mean[:], sg[:, 0:1], 1.0 / N_FULL)
        var = const.tile([P, 1], f32)
        nc.vector.tensor_scalar_mul(var[:], sg[:, 1:2], 1.0 / N_FULL)
        msq = const.tile([P, 1], f32)
        nc.vector.tensor_mul(msq[:], mean[:], mean[:])
        nc.vector.tensor_sub(var[:], var[:], msq[:])
        nc.vector.tensor_scalar_add(var[:], var[:], BN_EPS)
        nc.scalar.sqrt(var[:], var[:])
        s_t = const.tile([P, 1], f32)
        nc.vector.reciprocal(s_t[:], var[:])
        nc.vector.tensor_mul(s_t[:], gam_sb[:], s_t[:])
        t_t = const.tile([P, 1], f32)
        nc.vector.tensor_mul(t_t[:], mean[:], s_t[:])
        nc.vector.tensor_sub(t_t[:], bet_sb[:], t_t[:])

        # ---- epilogue: out = relu(agg*s + t) + x.  The BN affine is
        # per-feature (per-partition), so the whole node range is processed
        # in a few wide strips, written back into agg_out (bf16), then
        # stored with one large DMA.
        NSTRIP = 8
        SW = N_PAD // NSTRIP
        for s in range(NSTRIP):
            y = work.tile([P, SW], f32, tag="y")
            nc.scalar.activation(
                out=y[:],
                in_=agg_out[:, s * SW : (s + 1) * SW],
                func=AF.Relu,
                scale=s_t[:],
                bias=t_t[:],
            )
            nc.vector.tensor_tensor(
                out=agg_out[:, s * SW : (s + 1) * SW],
                in0=y[:],
                in1=xres_sb[:, s * SW : (s + 1) * SW],
                op=OP.add,
            )
        nc.sync.dma_start(out=out_d[:, :], in_=agg_out[:])

    nc.compile()
    return nc


# ---------------------------------------------------------------------------
# Host wrapper
# ---------------------------------------------------------------------------
def _in_maps(plan, x, W, gamma, beta):
    import ml_dtypes

    x = np.asarray(x, dtype=np.float32)
    xq = np.zeros((NX, P), dtype=ml_dtypes.bfloat16)
    xq[:N_FULL] = x.astype(ml_dtypes.bfloat16)
    wt = np.ascontiguousarray(np.asarray(W, dtype=np.float32).T)
    iota = np.tile(np.arange(P, dtype=ml_dtypes.bfloat16), (P, 1))
    gam = np.asarray(gamma, dtype=np.float32).reshape(P, 1)
    bet = np.asarray(beta, dtype=np.float32).reshape(P, 1)

    maps = []
    for k in range(N_CORES):
        xres = np.zeros((P, N_PAD), dtype=ml_dtypes.bfloat16)
        xres[:, :N_LOC] = x[k * N_LOC : (k + 1) * N_LOC].T.astype(ml_dtypes.bfloat16)
        m = dict(
            xq=xq,
            xres=xres,
            wt=wt,
            iota=iota,
            gam=gam,
            bet=bet,
            drel=np.ascontiguousarray(plan["drel"][k]),
            norm=np.ascontiguousarray(plan["norm"][k]),
            dinv2=np.ascontiguousarray(np.tile(plan["dinv2"][k], (P, 1))),
        )
        for c in range(N_CHUNK):
            m[f"idx{c}"] = np.ascontiguousarray(plan["idx"][c][k])
        maps.append(m)
    return maps


def run(x, edge_index, W, b, gamma, beta, trace=False):
    from concourse.bass_utils import run_bass_kernel_spmd

    plan = make_plan(np.asarray(edge_index))
    nc = build_nc(plan)
    maps = _in_maps(plan, x, W, gamma, beta)
    res = run_bass_kernel_spmd(nc, maps, core_ids=list(range(N_CORES)), trace=trace)
    out = np.concatenate(
        [res.results[k]["out"].astype(np.float32).T[:N_LOC] for k in range(N_CORES)],
        axis=0,
    )
    return out, res


def kernel(x, edge_index, W, b, gamma, beta):
    out, _ = run(x, edge_index, W, b, gamma, beta)
    return out
